# revision 10
# baseline (speedup 1.0000x reference)
"""Self-contained Trainium2 Bass kernel for the custom LSTM problem.

Problem: B=64, T=512, I=H=1024 LSTM variant (all gates sigmoid,
additive cell update c = f + c + i*g, h = o + tanh(c)).

Strategy: data-parallel over batch (8 rows per core, 8 cores).
Per core:
  Phase 1: xw[t,b,:] = x[b,t,:] @ W_re + b_re   (token-tiled GEMM, M=128)
  Phase 2: sequential recurrence; per step the 4-gate GEMM
    gates = sigmoid(xw_t + h @ U_re) with h kept in transposed
    [H,8] tiles as the stationary matmul operand; xw_t is injected
    into the same PSUM accumulation via an identity matmul.

Weight columns are rearranged host-side so column n*512 + g*128 + j
holds gate g (i,f,c,o), H-slice n, element j: each 512-column chunk
of the GEMM yields all 4 gates for one 128-wide H-slice, letting the
elementwise update pipeline behind the matmul stream.
"""

import numpy as np
import ml_dtypes

B, T, I, H = 64, 512, 1024, 1024
N_CORES = 8
BL = B // N_CORES            # batch rows per core
NK = I // 128                # contraction tiles
NCH = 8                      # 512-col chunks of the 4H gate dim
G4 = 4 * H                   # 4096

_COMPILED = {}


def _build(t_steps):
    import concourse.mybir as mybir
    from concourse import bacc
    from concourse.tile import TileContext

    f32 = mybir.dt.float32
    bf16 = mybir.dt.bfloat16
    AF = mybir.ActivationFunctionType

    nc = bacc.Bacc("TRN2", target_bir_lowering=False, debug=False,
                   num_devices=N_CORES)

    toks = t_steps * BL
    xT_e = nc.dram_tensor("xT", [I, toks], bf16, kind="ExternalInput").ap()
    W_e = nc.dram_tensor("W_re", [I, G4], bf16, kind="ExternalInput").ap()
    U_e = nc.dram_tensor("U_re", [I, G4], bf16, kind="ExternalInput").ap()
    b_e = nc.dram_tensor("b_re", [1, G4], bf16, kind="ExternalInput").ap()
    id8_e = nc.dram_tensor("id8", [BL, BL], bf16, kind="ExternalInput").ap()
    id8s_e = nc.dram_tensor("id8s", [128, BL], bf16,
                            kind="ExternalInput").ap()
    ones_e = nc.dram_tensor("ones1", [1, 128], bf16, kind="ExternalInput").ap()

    hs_e = nc.dram_tensor("hs", [t_steps, BL, H], bf16,
                          kind="ExternalOutput").ap()
    hc_e = nc.dram_tensor("hc", [2, BL, H], f32, kind="ExternalOutput").ap()

    with TileContext(nc) as tc:
        with (
            tc.tile_pool(name="dram", bufs=1, space="DRAM") as dpool,
            tc.tile_pool(name="const", bufs=1) as cpool,
            tc.tile_pool(name="upool", bufs=1) as upool,
            tc.tile_pool(name="xw", bufs=4) as xwpool,
            tc.tile_pool(name="gps", bufs=3, space="PSUM") as gpspool,
            tc.tile_pool(name="trps", bufs=2, space="PSUM") as trpspool,
            tc.tile_pool(name="gsb", bufs=3) as gpool,
            tc.tile_pool(name="tmp", bufs=4) as tmppool,
            tc.tile_pool(name="hsb", bufs=2) as hpool,
            tc.tile_pool(name="hT", bufs=2) as hTpool,
            tc.tile_pool(name="pers", bufs=1) as perspool,
        ):
            xw_d = dpool.tile([t_steps, BL, G4], bf16)

            id8_sb = cpool.tile([BL, BL], bf16)
            nc.sync.dma_start(out=id8_sb[:], in_=id8_e[:])
            id8s_sb = cpool.tile([128, BL], bf16)
            nc.sync.dma_start(out=id8s_sb[:], in_=id8s_e[:])
            ones_sb = cpool.tile([1, 128], bf16)
            nc.sync.dma_start(out=ones_sb[:], in_=ones_e[:])
            bias_sb = cpool.tile([1, G4], bf16)
            nc.sync.dma_start(out=bias_sb[:], in_=b_e[:])

            # resident U tiles (phase 2)
            u_sb = []
            for k in range(NK):
                ut = upool.tile([128, G4], bf16, tag=f"u{k}")
                nc.sync.dma_start(out=ut[:], in_=U_e[k * 128:(k + 1) * 128, :])
                u_sb.append(ut)

            # ---------------- phase 1: xw = x @ W + b ----------------
            with (
                tc.tile_pool(name="wpool", bufs=1) as wpool,
                tc.tile_pool(name="xt", bufs=3) as xtpool,
                tc.tile_pool(name="p1sb", bufs=3) as p1sb,
            ):
                w_sb = []
                for k in range(NK):
                    wt = wpool.tile([128, G4], bf16, tag=f"w{k}")
                    nc.sync.dma_start(out=wt[:],
                                      in_=W_e[k * 128:(k + 1) * 128, :])
                    w_sb.append(wt)

                n_mtiles = toks // 128
                tsteps_per_mtile = 128 // BL
                for m in range(n_mtiles):
                    xts = []
                    for k in range(NK):
                        xt = xtpool.tile([128, 128], bf16, tag=f"xt{k}",
                                         name=f"xt{k}")
                        nc.sync.dma_start(
                            out=xt[:],
                            in_=xT_e[k * 128:(k + 1) * 128,
                                     m * 128:(m + 1) * 128])
                        xts.append(xt)
                    for n in range(NCH):
                        ps = gpspool.tile([128, 512], f32, tag="g", name="ps1")
                        for k in range(NK):
                            nc.tensor.matmul(ps[:], xts[k][:],
                                             w_sb[k][:, n * 512:(n + 1) * 512],
                                             start=(k == 0), stop=False)
                        nc.tensor.matmul(ps[:], ones_sb[:],
                                         bias_sb[:, n * 512:(n + 1) * 512],
                                         start=False, stop=True)
                        xw_sb = p1sb.tile([128, 512], bf16)
                        nc.scalar.activation(xw_sb[:], ps[:], AF.Copy)
                        t0 = m * tsteps_per_mtile
                        nc.sync.dma_start(
                            out=xw_d[t0:t0 + tsteps_per_mtile, :,
                                     n * 512:(n + 1) * 512],
                            in_=xw_sb[:])

            # ---------------- phase 2: recurrence ----------------
            # chunk n = q*4 + r (q = n // 4, r = n % 4) covers H-slice n and
            # computes on partitions [32r, 32r+8) (PE col-group r), free
            # block q. Halves h = r // 2 share one [128,1024] PSUM tile and
            # one merged sigmoid/tanh over partitions [64h, 64h+40) — the
            # 24-partition gaps hold garbage that is never read back.
            c_sb = perspool.tile([128, 256], f32, tag="c")
            nc.any.memset(c_sb[:], 0.0)

            hT_cur = []
            for k in range(NK):
                t_ = hTpool.tile([128, BL], bf16, tag=f"hT{k}", name=f"hT{k}")
                nc.any.memset(t_[:], 0.0)
                hT_cur.append(t_)

            KORDER = [0, 4, 1, 5, 2, 6, 3, 7]

            def hrange(ap, h, lo, hi):
                return ap[64 * h:64 * h + 40, lo:hi]

            def hview(ap, h, lo, hi):
                return ap[64 * h:64 * h + 40, :].rearrange(
                    "p (q x) -> p q x", q=2)[:, :, lo:hi]

            for t in range(t_steps):
                xw_sb = xwpool.tile([BL, G4], bf16)
                nc.sync.dma_start(out=xw_sb[:], in_=xw_d[t, :, :])
                h_sb = hpool.tile([128, 256], bf16)
                hT_new = [hTpool.tile([128, BL], bf16, tag=f"hT{k}",
                                      name=f"hTn{k}")
                          for k in range(NK)]
                half_ps = [gpspool.tile([128, 1024], f32, tag="g",
                                        name=f"ps{h}") for h in range(2)]
                for n in range(NCH):
                    q, r = divmod(n, 4)
                    out_sl = half_ps[r // 2][32 * r:32 * r + BL,
                                             q * 512:(q + 1) * 512]
                    for j, k in enumerate(KORDER):
                        nc.tensor.matmul(out_sl, hT_cur[k][:],
                                         u_sb[k][:, n * 512:(n + 1) * 512],
                                         tile_position=(0, 32 * r),
                                         start=(j == 0), stop=False)
                    nc.tensor.matmul(out_sl, id8_sb[:],
                                     xw_sb[:, n * 512:(n + 1) * 512],
                                     tile_position=(0, 32 * r),
                                     start=False, stop=True)
                for h in range(2):
                    g_sb = gpool.tile([128, 1024], f32, tag="gs",
                                      name=f"g{h}")
                    nc.scalar.activation(hrange(g_sb, h, 0, 1024),
                                         hrange(half_ps[h], h, 0, 1024),
                                         AF.Sigmoid)
                    ig = tmppool.tile([128, 256], f32, tag="ig",
                                      name=f"ig{h}")
                    nc.vector.tensor_mul(hview(ig, h, 0, 128),
                                         hview(g_sb, h, 0, 128),
                                         hview(g_sb, h, 256, 384))
                    cv = hview(c_sb, h, 0, 128)
                    nc.vector.tensor_add(cv, cv, hview(g_sb, h, 128, 256))
                    nc.vector.tensor_add(cv, cv, hview(ig, h, 0, 128))
                    th = tmppool.tile([128, 256], f32, tag="th",
                                      name=f"th{h}")
                    nc.scalar.activation(hrange(th, h, 0, 256),
                                         hrange(c_sb, h, 0, 256), AF.Tanh)
                    nc.gpsimd.tensor_add(hview(h_sb, h, 0, 128),
                                         hview(g_sb, h, 384, 512),
                                         hview(th, h, 0, 128))
                    for rr in range(2):
                        r = 2 * h + rr
                        for q in range(2):
                            n = q * 4 + r
                            tp = trpspool.tile([128, BL], bf16, tag="tp",
                                               name=f"tp{n}")
                            nc.tensor.transpose(
                                tp[:],
                                h_sb[32 * r:32 * r + BL,
                                     q * 128:(q + 1) * 128],
                                id8s_sb[32 * r:32 * r + BL, :],
                                tile_position=(32 * r, 0))
                            nc.vector.tensor_copy(hT_new[n][:], tp[:])
                for r in range(4):
                    nc.sync.dma_start(
                        out=hs_e[t, :, :].rearrange(
                            "b (q rr j) -> b rr q j", q=2, rr=4)[:, r],
                        in_=h_sb[32 * r:32 * r + BL, :].rearrange(
                            "b (q j) -> b q j", q=2))
                hT_cur = hT_new

            hf = perspool.tile([128, 256], f32, tag="hf")
            nc.vector.tensor_copy(hf[:], h_sb[:])
            for r in range(4):
                nc.sync.dma_start(
                    out=hc_e[0, :, :].rearrange(
                        "b (q rr j) -> b rr q j", q=2, rr=4)[:, r],
                    in_=hf[32 * r:32 * r + BL, :].rearrange(
                        "b (q j) -> b q j", q=2))
                nc.sync.dma_start(
                    out=hc_e[1, :, :].rearrange(
                        "b (q rr j) -> b rr q j", q=2, rr=4)[:, r],
                    in_=c_sb[32 * r:32 * r + BL, :].rearrange(
                        "b (q j) -> b q j", q=2))

    nc.compile()
    return nc


def _prep_inputs(x, w_i, u_i, b_i, w_f, u_f, b_f, w_c, u_c, b_c,
                 w_o, u_o, b_o, t_steps):
    """Host-side shard + rearrange. Returns per-core input maps."""
    bf = ml_dtypes.bfloat16

    # column rearrangement: col n*512 + g*128 + j  <-  gate g col n*128 + j
    def rearrange_cols(mats):  # mats: list of 4 [I, H] (or [H]) gate arrays
        stacked = np.stack(mats, axis=0)  # [4, I, H] or [4, H]
        if stacked.ndim == 3:
            v = stacked.reshape(4, I, NCH, 128)          # g, i, n, j
            v = np.transpose(v, (1, 2, 0, 3))            # i, n, g, j
            return np.ascontiguousarray(v.reshape(I, G4))
        v = stacked.reshape(4, NCH, 128)
        v = np.transpose(v, (1, 0, 2))
        return np.ascontiguousarray(v.reshape(1, G4))

    W_re = rearrange_cols([w_i, w_f, w_c, w_o]).astype(bf)
    U_re = rearrange_cols([u_i, u_f, u_c, u_o]).astype(bf)
    b_re = rearrange_cols([b_i, b_f, b_c, b_o]).astype(bf)
    id8 = np.eye(BL, dtype=np.float32).astype(bf)
    id8s = np.zeros((128, BL), dtype=np.float32)
    for r in range(4):
        id8s[32 * r:32 * r + BL, :] = np.eye(BL)
    id8s = id8s.astype(bf)
    ones1 = np.ones((1, 128), dtype=np.float32).astype(bf)

    in_maps = []
    for c in range(N_CORES):
        xc = x[c * BL:(c + 1) * BL, :t_steps, :]         # [BL, T, I]
        xT = np.transpose(xc, (2, 1, 0)).reshape(I, t_steps * BL)
        in_maps.append({
            "xT": np.ascontiguousarray(xT).astype(bf),
            "W_re": W_re, "U_re": U_re, "b_re": b_re,
            "id8": id8, "id8s": id8s, "ones1": ones1,
        })
    return in_maps


def run(x, w_i, u_i, b_i, w_f, u_f, b_f, w_c, u_c, b_c, w_o, u_o, b_o,
        t_steps=T):
    from concourse.bass_utils import run_bass_kernel_spmd

    if t_steps not in _COMPILED:
        _COMPILED[t_steps] = _build(t_steps)
    nc = _COMPILED[t_steps]

    in_maps = _prep_inputs(x, w_i, u_i, b_i, w_f, u_f, b_f, w_c, u_c, b_c,
                           w_o, u_o, b_o, t_steps)
    res = run_bass_kernel_spmd(nc, in_maps, list(range(N_CORES)))

    h_t = np.empty((B, H), np.float32)
    c_t = np.empty((B, H), np.float32)
    hidden = np.empty((B, t_steps, H), np.float32)
    for c in range(N_CORES):
        r = res.results[c]
        h_t[c * BL:(c + 1) * BL] = r["hc"][0]
        c_t[c * BL:(c + 1) * BL] = r["hc"][1]
        hidden[c * BL:(c + 1) * BL] = np.transpose(
            r["hs"].astype(np.float32), (1, 0, 2))
    return h_t, c_t, hidden


def kernel(**inputs):
    args = {k: np.asarray(v) for k, v in inputs.items()}
    return run(args["x"], args["w_i"], args["u_i"], args["b_i"],
               args["w_f"], args["u_f"], args["b_f"],
               args["w_c"], args["u_c"], args["b_c"],
               args["w_o"], args["u_o"], args["b_o"])
